# revision 17
# baseline (speedup 1.0000x reference)
"""Multi-head causal attention (B=2, S=2048, E=1024, H=16, D=64) on 8 TRN2
NeuronCores. Sharding: batch (2) x head-groups (4 heads each) -> 8 cores.
Each core computes Q/K/V projections for its 4 heads, RoPE, flash-style
causal attention, and a partial output projection (its head columns of Wo);
the host sums the 4 partials per batch.

Layout notes:
- Q/K are produced directly in transposed [dims, S] layout by making the
  weight the stationary matmul operand. Per head, dims are permuted to
  [evens(32); odds(32)] so RoPE pair-halves are contiguous partition slices;
  the same permutation is applied to Q and K so scores are unchanged.
- Weight columns are arranged so psum chunk E holds the even-halves of all
  4 heads (rows 32h..32h+31 = head h) and chunk O the odd-halves. RoPE is
  then 6 full-width [128,512] DVE ops per S-chunk reading the two psums.
- Scores are computed transposed (scoresT[k,q]) so the AV matmul can use
  V as the stationary operand in natural [S, dims] layout; a ones column
  appended to V yields the softmax denominators in psum row 64.
- Softmax normalization: reciprocal (DVE) -> partition_broadcast (GPSIMD,
  attn ucode library) -> multiply during the psum->SBUF drain (DVE).
- Projections run as float32r (full-rate fp32); attention math in bf16
  with fp32 psum accumulation.
"""

import sys

if "/opt/trn_rl_repo" not in sys.path:
    sys.path.insert(0, "/opt/trn_rl_repo")

import numpy as np
import ml_dtypes

B, S, E, H = 2, 2048, 1024, 16
D = E // H          # 64
HPC = 4             # heads per core
NCORES = 8
NE = E // 128       # 8 contraction chunks
NQ = S // 512       # 4 q-chunks
NK = S // 128       # 16 k-blocks
ROPE_BASE = 10000.0
ATTN_SCALE = 1.0 / np.sqrt(E)


def build_bass():
    import concourse.bass as bass
    import concourse.mybir as mybir
    from concourse import bacc
    from concourse import library_config
    from concourse.tile import TileContext

    F32 = mybir.dt.float32
    BF16 = mybir.dt.bfloat16
    F32R = mybir.dt.float32r
    Exp = mybir.ActivationFunctionType.Exp

    nc = bacc.Bacc()

    xT_e = nc.declare_dram_parameter("xT", [E, S], F32R, isOutput=False)
    wq_e = nc.declare_dram_parameter("wq", [E, 2, 128], F32R, isOutput=False)
    wk_e = nc.declare_dram_parameter("wk", [E, 2, 128], F32R, isOutput=False)
    wv_e = nc.declare_dram_parameter("wv", [E, 256], F32R, isOutput=False)
    wo_e = nc.declare_dram_parameter("wo", [256, E], BF16, isOutput=False)
    cs_e = nc.declare_dram_parameter("cs", [128, S], BF16, isOutput=False)
    sn_e = nc.declare_dram_parameter("sn", [128, S], BF16, isOutput=False)
    dm_e = nc.declare_dram_parameter("dmask", [4, 128, 512], BF16, isOutput=False)
    out_e = nc.declare_dram_parameter("out", [S, E], F32, isOutput=True)

    nc.gpsimd.load_library(library_config.attn)

    with TileContext(nc) as tc:
        with (
            tc.tile_pool(name="wpool", bufs=1) as wpool,
            tc.tile_pool(name="xpool", bufs=1) as xpool,
            tc.tile_pool(name="qk", bufs=1) as qkpool,
            tc.tile_pool(name="vpool", bufs=1) as vpool,
            tc.tile_pool(name="epool", bufs=6) as epool,
            tc.tile_pool(name="rtmp", bufs=4) as rtmp,
            tc.tile_pool(name="atp", bufs=1) as atpool,
            tc.tile_pool(name="ypool", bufs=3) as ypool,
            tc.tile_pool(name="npool", bufs=2) as npool,
            tc.tile_pool(name="psA", bufs=2, space="PSUM") as psA,
            tc.tile_pool(name="psS", bufs=1, space="PSUM") as psS,
            tc.tile_pool(name="psO", bufs=1, space="PSUM") as psO,
        ):
            # ---- static inputs (x first: it gates the projection matmuls) ----
            x_t = []
            for e in range(NE):
                xt = xpool.tile([128, S], F32R, tag=f"x{e}")
                nc.sync.dma_start(xt[:], xT_e[128 * e : 128 * (e + 1), :])
                x_t.append(xt)

            wq_sb = wpool.tile([128, NE, 2, 128], F32R, tag="wq")
            nc.sync.dma_start(wq_sb[:], wq_e.rearrange("(ne p) o m -> p ne o m", p=128))
            wk_sb = wpool.tile([128, NE, 2, 128], F32R, tag="wk")
            nc.sync.dma_start(wk_sb[:], wk_e.rearrange("(ne p) o m -> p ne o m", p=128))
            wv_sb = wpool.tile([128, NE, 256], F32R, tag="wv")
            nc.sync.dma_start(wv_sb[:], wv_e.rearrange("(ne p) m -> p ne m", p=128))
            cs_sb = wpool.tile([128, S], BF16, tag="cs")
            nc.sync.dma_start(cs_sb[:], cs_e[:])
            sn_sb = wpool.tile([128, S], BF16, tag="sn")
            nc.sync.dma_start(sn_sb[:], sn_e[:])
            dm_sb = wpool.tile([128, 4, 512], BF16, tag="dm")
            nc.sync.dma_start(dm_sb[:], dm_e.rearrange("r p c -> p r c"))
            wo_sb = wpool.tile([128, 2, E], BF16, tag="wo")
            nc.sync.dma_start(wo_sb[:], wo_e.rearrange("(c p) e -> p c e", p=128))

            # ---- projections + RoPE -------------------------------------------
            # qe_t[j]: [128, 512] bf16, rows 32h..32h+31 = head h even dims
            qe_t, qo_t, ke_t, ko_t = [], [], [], []
            for w_sb, et_list, ot_list, nm in (
                (wq_sb, qe_t, qo_t, "q"),
                (wk_sb, ke_t, ko_t, "k"),
            ):
                for j in range(NQ):
                    sl = slice(512 * j, 512 * (j + 1))
                    pe_ps = psA.tile([128, 512], F32, tag="pp")
                    po_ps = psA.tile([128, 512], F32, tag="pp")
                    for e in range(NE):
                        nc.tensor.matmul(
                            pe_ps[:], w_sb[:, e, 0, :],
                            x_t[e][:, sl],
                            start=(e == 0), stop=(e == NE - 1))
                    for e in range(NE):
                        nc.tensor.matmul(
                            po_ps[:], w_sb[:, e, 1, :],
                            x_t[e][:, sl],
                            start=(e == 0), stop=(e == NE - 1))
                    # drain psums to bf16 once, then RoPE in 4x bf16 DVE mode
                    pe_sb = rtmp.tile([128, 512], BF16, tag="pe_sb")
                    po_sb = rtmp.tile([128, 512], BF16, tag="po_sb")
                    nc.vector.tensor_copy(pe_sb[:], pe_ps[:])
                    nc.vector.tensor_copy(po_sb[:], po_ps[:])
                    t1 = rtmp.tile([128, 512], BF16, tag="t1")
                    t2 = rtmp.tile([128, 512], BF16, tag="t2")
                    t3 = rtmp.tile([128, 512], BF16, tag="t3")
                    t4 = rtmp.tile([128, 512], BF16, tag="t4")
                    nc.vector.tensor_mul(t1[:], pe_sb[:], cs_sb[:, sl])
                    nc.vector.tensor_mul(t2[:], po_sb[:], sn_sb[:, sl])
                    nc.vector.tensor_mul(t3[:], pe_sb[:], sn_sb[:, sl])
                    nc.vector.tensor_mul(t4[:], po_sb[:], cs_sb[:, sl])
                    et = qkpool.tile([128, 512], BF16, tag=f"{nm}e{j}")
                    ot = qkpool.tile([128, 512], BF16, tag=f"{nm}o{j}")
                    nc.vector.tensor_sub(et[:], t1[:], t2[:])
                    nc.vector.tensor_add(ot[:], t3[:], t4[:])
                    et_list.append(et)
                    ot_list.append(ot)

            # V: natural [S, dims] layout with a ones column per head (65 wide)
            v_t = []
            for i in range(NK):
                pv = psA.tile([128, 256], F32, tag="pp")
                for e in range(NE):
                    nc.tensor.matmul(
                        pv[:], x_t[e][:, 128 * i : 128 * (i + 1)],
                        wv_sb[:, e, :],
                        start=(e == 0), stop=(e == NE - 1))
                vt = vpool.tile([128, 4, 65], BF16, tag=f"v{i}")
                nc.vector.tensor_copy(
                    vt[:, :, 0:64], pv[:].rearrange("p (h d) -> p h d", d=64))
                nc.vector.memset(vt[:, :, 64], 1.0)
                v_t.append(vt)

            # ---- attention -----------------------------------------------------
            for jq in range(NQ):
                po = [psO.tile([65, 512], F32, tag=f"o{h}", name=f"po{h}")
                      for h in range(HPC)]
                nblk = 4 * jq + 4
                for i in range(nblk):
                    r = i - 4 * jq
                    q0 = 128 * max(r, 0)
                    jsl = slice(128 * (i % 4), 128 * (i % 4) + 128)
                    for h in range(HPC):
                        hr = slice(32 * h, 32 * h + 32)
                        ss = psS.tile([128, 512], F32, tag=f"s{h % 2}")
                        nc.tensor.matmul(
                            ss[:, q0:512], ke_t[i // 4][hr, jsl],
                            qe_t[jq][hr, q0:512],
                            start=True, stop=False, tile_position=(32 * h, 0))
                        nc.tensor.matmul(
                            ss[:, q0:512], ko_t[i // 4][hr, jsl],
                            qo_t[jq][hr, q0:512],
                            start=False, stop=True, tile_position=(32 * h, 0))
                        et = epool.tile([128, 512], BF16, tag="e")
                        nc.scalar.activation(
                            et[:, q0:512], ss[:, q0:512], Exp, scale=ATTN_SCALE)
                        if r >= 0:
                            nc.vector.tensor_mul(
                                et[:, q0:512], et[:, q0:512], dm_sb[:, r, q0:512])
                        nc.tensor.matmul(
                            po[h][:, q0:512], v_t[i][:, h, :], et[:, q0:512],
                            start=(i == 0), stop=(i == nblk - 1))

                # normalize: at[h] = po[h][0:64] * (1 / po[h][64]) -> bf16
                at_c = [atpool.tile([128, 512], BF16, tag=f"at{c}_{jq}",
                                    name=f"at{c}_{jq}") for c in range(2)]
                for h in range(HPC):
                    ao = npool.tile([65, 512], F32, tag="ao")
                    nc.vector.tensor_copy(ao[:], po[h][:])
                    rt = npool.tile([1, 512], BF16, tag="rt")
                    with nc.allow_low_precision(reason="softmax denom recip in bf16"):
                        nc.vector.reciprocal(rt[:], ao[64:65, :])
                    # broadcast recip row across 64 partitions (gpsimd ucode)
                    bt = npool.tile([64, 512], BF16, tag="bt")
                    nc.gpsimd.partition_broadcast(bt[:], rt[:])
                    ro = 64 * (h % 2)
                    nc.vector.tensor_mul(
                        at_c[h // 2][ro : ro + 64, :], ao[0:64, :], bt[:])

                # output projection for this q range
                for qb in range(4):
                    lsl = slice(128 * qb, 128 * qb + 128)
                    orow = 128 * (4 * jq + qb)
                    for ec in range(2):
                        esl = slice(512 * ec, 512 * (ec + 1))
                        yp = psA.tile([128, 512], F32, tag="pp")
                        for c in range(2):
                            nc.tensor.matmul(
                                yp[:], at_c[c][:, lsl], wo_sb[:, c, esl],
                                start=(c == 0), stop=(c == 1))
                        ys = ypool.tile([128, 512], F32, tag="y")
                        nc.vector.tensor_copy(ys[:], yp[:])
                        nc.sync.dma_start(
                            out_e[orow : orow + 128, esl], ys[:])
    nc.finalize()
    return nc


def host_inputs(x, Wq, Wk, Wv, Wo):
    """Build the 8 per-core input maps (numpy, host-side shard/permute)."""
    perm = np.concatenate([np.arange(0, D, 2), np.arange(1, D, 2)])  # evens;odds
    d2 = D // 2
    theta = 1.0 / (ROPE_BASE ** (np.arange(d2, dtype=np.float64) * 2.0 / D))
    pos = np.arange(S, dtype=np.float64)
    ang = pos[None, :] * theta[:, None]              # [32, S]
    cs = np.tile(np.cos(ang), (4, 1)).astype(ml_dtypes.bfloat16)  # [128, S]
    sn = np.tile(np.sin(ang), (4, 1)).astype(ml_dtypes.bfloat16)

    dm = np.zeros((4, 128, 512), dtype=np.float32)
    k_idx = np.arange(128)[:, None]
    c_idx = np.arange(512)[None, :]
    for r in range(4):
        dm[r] = (k_idx <= c_idx - 128 * r).astype(np.float32)
    dm = dm.astype(ml_dtypes.bfloat16)

    in_maps = []
    for c in range(NCORES):
        b, g = divmod(c, HPC)
        heads = [HPC * g + t for t in range(HPC)]
        # evens chunk cols: head-major, 32 even dims each; odds chunk likewise
        ecols = np.concatenate([D * h + perm[:d2] for h in heads])
        ocols = np.concatenate([D * h + perm[d2:] for h in heads])
        vcols = np.concatenate([D * h + np.arange(D) for h in heads])
        wq = np.stack([Wq.T[:, ecols], Wq.T[:, ocols]], axis=1)  # [E, 2, 128]
        wk = np.stack([Wk.T[:, ecols], Wk.T[:, ocols]], axis=1)
        wv = Wv.T[:, vcols]                                      # [E, 256]
        wo = Wo[:, vcols].T.astype(ml_dtypes.bfloat16)           # [256, E]
        in_maps.append({
            "xT": np.ascontiguousarray(x[b].T).astype(np.float32),
            "wq": np.ascontiguousarray(wq).astype(np.float32),
            "wk": np.ascontiguousarray(wk).astype(np.float32),
            "wv": np.ascontiguousarray(wv).astype(np.float32),
            "wo": np.ascontiguousarray(wo),
            "cs": cs, "sn": sn, "dmask": dm,
        })
    return in_maps


_CACHED = {}


def kernel(x, Wq, Wk, Wv, Wo):
    from concourse.bass_utils import run_bass_kernel_spmd

    if "nc" not in _CACHED:
        _CACHED["nc"] = build_bass()
    nc = _CACHED["nc"]
    in_maps = host_inputs(
        np.asarray(x, dtype=np.float32), np.asarray(Wq, dtype=np.float32),
        np.asarray(Wk, dtype=np.float32), np.asarray(Wv, dtype=np.float32),
        np.asarray(Wo, dtype=np.float32))
    res = run_bass_kernel_spmd(nc, in_maps, core_ids=list(range(NCORES)))
    y = np.empty((B, S, E), dtype=np.float32)
    for b in range(B):
        y[b] = sum(res.results[HPC * b + g]["out"] for g in range(HPC))
    return y
